# revision 1
# baseline (speedup 1.0000x reference)
"""MixER MoE-hypernetwork kernel for 8 Trainium2 NeuronCores.

Expert-parallel: core e handles expert e (NEXP == n_cores == 8).
Per core:
  phase 1: deltaT[blk, env, col] = ctx @ H^T   (hypernet; H pre-permuted on
           host so delta chunks land directly in transposed weight layout,
           stored in 2048-column blocks, bf16)
  phase 2: per env: fW = W + delta chunk (DVE adds, bf16), 4-layer MLP in
           feature-major layout (features on partitions, points on free dim,
           bf16 operands / fp32 PSUM accumulate), swish via a single ACT Silu
           op per tile (beta folded via host weight scaling), gate applied in
           the DVE epilogue.
Host: computes gate softmax, transposes y, permutes/scales/casts H, and sums
the 8 per-expert partial outputs.
"""
import os
import numpy as np
import ml_dtypes

import concourse.bass as bass
import concourse.bacc as bacc
import concourse.tile as tile
from concourse import mybir
from concourse.bass_utils import run_bass_kernel_spmd

# ---- problem dims (hardcoded; must match the grader's setup_inputs) ----
DATA, WIDTH, CTXD, NEXP, ENVS, NPTS = 64, 256, 128, 8, 16, 2048
SIZES = [WIDTH * DATA, WIDTH, WIDTH * WIDTH, WIDTH, WIDTH * WIDTH, WIDTH,
         DATA * WIDTH, DATA]
OFFS = np.cumsum([0] + SIZES)
NET_USED = int(OFFS[-1])          # 164672
BLK = 2048
NBLK = 81
NETPAD = NBLK * BLK               # 165888

# new (device) layout offsets: [W1T | W2T | W3T | W4T | b1 | b2 | b3 | b4]
O_W1, O_W2, O_W3, O_W4 = 0, 16384, 81920, 147456
O_B1, O_B2, O_B3, O_B4 = 163840, 164096, 164352, 164608

F32 = mybir.dt.float32
BF16 = mybir.dt.bfloat16
BF16_NP = ml_dtypes.bfloat16

N_CORES = 8
TRACE = os.environ.get("MIXER_TRACE", "0") == "1"

if TRACE:
    # The agent image's antenv lacks axon_hooks, so run_bass_kernel_spmd's
    # trace path can't find the NTFF profile hook. Shim it with the ctypes
    # hook factory that trn_boot ships. Profiling-only; inert when TRACE=0.
    try:
        from antenv.axon_hooks import get_axon_ntff_profile_hook  # noqa: F401
    except ImportError:
        import sys as _sys
        import types as _types
        try:
            from trn_agent_boot.trn_boot import _ntff_profile_via_ctypes
            _hook = _ntff_profile_via_ctypes("/opt/axon/libaxon_pjrt.so")
            import antenv as _antenv
            _mod = _types.ModuleType("antenv.axon_hooks")
            _mod.get_axon_ntff_profile_hook = lambda: _hook
            _mod.set_axon_ntff_profile_hook = lambda h: None
            _sys.modules["antenv.axon_hooks"] = _mod
            _antenv.axon_hooks = _mod
        except Exception as _e:  # pragma: no cover - profiling is best-effort
            print(f"NTFF hook shim failed: {_e}")

LAST_RESULTS = None  # BassKernelResults of the most recent run (for test.py)

_NC_CACHE = {}
_PERM_CACHE = {}


# --------------------------------------------------------------------------
# host-side preprocessing
# --------------------------------------------------------------------------
def _build_perm():
    """perm[new_row] = old_row of H's NET axis."""
    if "perm" in _PERM_CACHE:
        return _PERM_CACHE["perm"]
    perm = np.zeros(NET_USED, dtype=np.int64)
    # W1: orig OFFS[0] + w*DATA+d  -> new O_W1 + d*WIDTH+w   ([64,256] = fW1T)
    d, w = np.meshgrid(np.arange(DATA), np.arange(WIDTH), indexing="ij")
    perm[O_W1 + (d * WIDTH + w).ravel()] = OFFS[0] + (w * DATA + d).ravel()
    # W2/W3: orig + v*WIDTH+w (v,w) -> new + w*WIDTH+v  ([256,256] = fW2T)
    w2, v2 = np.meshgrid(np.arange(WIDTH), np.arange(WIDTH), indexing="ij")
    perm[O_W2 + (w2 * WIDTH + v2).ravel()] = OFFS[2] + (v2 * WIDTH + w2).ravel()
    perm[O_W3 + (w2 * WIDTH + v2).ravel()] = OFFS[4] + (v2 * WIDTH + w2).ravel()
    # W4: orig + d*WIDTH+w (d,w) -> new + w*DATA+d  ([256,64] = fW4T)
    d4, w4 = np.meshgrid(np.arange(DATA), np.arange(WIDTH), indexing="ij")
    perm[O_W4 + (w4 * DATA + d4).ravel()] = OFFS[6] + (d4 * WIDTH + w4).ravel()
    perm[O_B1:O_B1 + WIDTH] = OFFS[1] + np.arange(WIDTH)
    perm[O_B2:O_B2 + WIDTH] = OFFS[3] + np.arange(WIDTH)
    perm[O_B3:O_B3 + WIDTH] = OFFS[5] + np.arange(WIDTH)
    perm[O_B4:O_B4 + DATA] = OFFS[7] + np.arange(DATA)
    _PERM_CACHE["perm"] = perm
    return perm


def _build_scale(beta_e):
    ib = np.float32(1.0 / beta_e)
    scale = np.ones(NET_USED, dtype=np.float32)
    scale[O_W2:O_W2 + WIDTH * WIDTH] = ib
    scale[O_W3:O_W3 + WIDTH * WIDTH] = ib
    scale[O_W4:O_W4 + WIDTH * DATA] = ib
    scale[O_B1:O_B1 + WIDTH] = beta_e
    scale[O_B2:O_B2 + WIDTH] = beta_e
    scale[O_B3:O_B3 + WIDTH] = beta_e
    return scale


def _prep_inputs(y, ctx, W, b, H, G, beta):
    """Returns in_maps: one dict per core."""
    perm = _build_perm()

    # gate softmax on host (tiny)
    logits = ctx.astype(np.float32) @ G.astype(np.float32).T      # [B, E]
    m = logits.max(-1, keepdims=True)
    eg = np.exp(logits - m)
    gate = (eg / eg.sum(-1, keepdims=True)).astype(np.float32)

    yT = np.ascontiguousarray(y.transpose(0, 2, 1)).astype(BF16_NP)
    ctxT = np.ascontiguousarray(ctx.T).astype(BF16_NP)            # [128, 16]

    in_maps = []
    for e in range(NEXP):
        be = float(beta[e])
        scale = _build_scale(be)
        Hp = H[e][perm] * scale[:, None]                          # [NET_USED, 128]
        htf = np.zeros((CTXD, NETPAD), dtype=BF16_NP)
        htf[:, :NET_USED] = Hp.T.astype(BF16_NP)
        # blocked layout: [NBLK, 128, BLK], each block contiguous in DRAM
        ht = np.ascontiguousarray(
            htf.reshape(CTXD, NBLK, BLK).transpose(1, 0, 2))

        w1t = np.ascontiguousarray(W[0][e].T).astype(BF16_NP)     # [64, 256]
        w2t = np.ascontiguousarray(
            (W[1][e].T / be).reshape(2, 128, WIDTH).transpose(1, 0, 2)
            .reshape(128, 2 * WIDTH)).astype(BF16_NP)             # [128, 512]
        w3t = np.ascontiguousarray(
            (W[2][e].T / be).reshape(2, 128, WIDTH).transpose(1, 0, 2)
            .reshape(128, 2 * WIDTH)).astype(BF16_NP)
        w4t = np.ascontiguousarray(
            (W[3][e].T / be).reshape(2, 128, DATA).transpose(1, 0, 2)
            .reshape(128, 2 * DATA)).astype(BF16_NP)              # [128, 128]
        b1d = np.ascontiguousarray((b[0][e] * be).reshape(2, 128).T,
                                   dtype=np.float32)              # [128, 2]
        b2d = np.ascontiguousarray((b[1][e] * be).reshape(2, 128).T,
                                   dtype=np.float32)
        b3d = np.ascontiguousarray((b[2][e] * be).reshape(2, 128).T,
                                   dtype=np.float32)
        b4d = np.ascontiguousarray(b[3][e].reshape(DATA, 1), dtype=np.float32)

        in_maps.append({
            "ht": ht, "ctxt": ctxT, "yt": yT,
            "w1t": w1t, "w2t": w2t, "w3t": w3t, "w4t": w4t,
            "b1": b1d, "b2": b2d, "b3": b3d, "b4": b4d,
            "gate": np.ascontiguousarray(gate[:, e]),             # [16]
            "beta": np.array([be], dtype=np.float32),
        })
    return in_maps


# --------------------------------------------------------------------------
# device kernel (SPMD program, one expert per core)
# --------------------------------------------------------------------------
def _build_nc():
    if "nc" in _NC_CACHE:
        return _NC_CACHE["nc"]
    nc = bacc.Bacc()
    P = 128

    ht = nc.declare_dram_parameter("ht", [NBLK, CTXD, BLK], BF16, isOutput=False)
    ctxt = nc.declare_dram_parameter("ctxt", [CTXD, ENVS], BF16, isOutput=False)
    yt = nc.declare_dram_parameter("yt", [ENVS, DATA, NPTS], BF16, isOutput=False)
    w1t = nc.declare_dram_parameter("w1t", [DATA, WIDTH], BF16, isOutput=False)
    w2t = nc.declare_dram_parameter("w2t", [P, 2 * WIDTH], BF16, isOutput=False)
    w3t = nc.declare_dram_parameter("w3t", [P, 2 * WIDTH], BF16, isOutput=False)
    w4t = nc.declare_dram_parameter("w4t", [P, 2 * DATA], BF16, isOutput=False)
    b1 = nc.declare_dram_parameter("b1", [P, 2], F32, isOutput=False)
    b2 = nc.declare_dram_parameter("b2", [P, 2], F32, isOutput=False)
    b3 = nc.declare_dram_parameter("b3", [P, 2], F32, isOutput=False)
    b4 = nc.declare_dram_parameter("b4", [DATA, 1], F32, isOutput=False)
    gate = nc.declare_dram_parameter("gate", [ENVS], F32, isOutput=False)
    beta = nc.declare_dram_parameter("beta", [1], F32, isOutput=False)
    out = nc.declare_dram_parameter("out", [ENVS, DATA, NPTS], F32, isOutput=True)
    DBG = os.environ.get("MIXER_DEBUG", "0") == "1"
    if DBG:
        dbg_delta = nc.declare_dram_parameter("dbg_delta", [ENVS, BLK], BF16, isOutput=True)
        dbg_dw2 = nc.declare_dram_parameter("dbg_dw2", [P, 2 * WIDTH], BF16, isOutput=True)
        dbg_fw2 = nc.declare_dram_parameter("dbg_fw2", [P, 2 * WIDTH], BF16, isOutput=True)
        dbg_h1 = nc.declare_dram_parameter("dbg_h1", [P, NPTS], BF16, isOutput=True)
        dbg_y = nc.declare_dram_parameter("dbg_y", [DATA, NPTS], BF16, isOutput=True)

    SILU = mybir.ActivationFunctionType.Silu
    MULT, ADD = mybir.AluOpType.mult, mybir.AluOpType.add

    def _bcast(handle, parts):
        """Broadcast a 1-D DRAM tensor across `parts` partitions."""
        ap = handle[:]
        return bass.AP(tensor=ap.tensor, offset=ap.offset,
                       ap=[[0, parts]] + list(ap.ap))

    with tile.TileContext(nc) as tc:
        with tc.tile_pool(name="dram", bufs=1, space="DRAM") as dram_pool, \
             tc.tile_pool(name="const", bufs=1) as const:
            # bf16 blocked delta: [blk, env, col]
            deltaT = dram_pool.tile([NBLK, ENVS, BLK], BF16)

            # constants loaded once
            ctx_sb = const.tile([CTXD, ENVS], BF16)
            nc.sync.dma_start(out=ctx_sb, in_=ctxt[:, :])
            beta_sb = const.tile([P, 1], F32)
            nc.sync.dma_start(out=beta_sb, in_=_bcast(beta, P))
            gate_sb = const.tile([DATA, ENVS], F32)
            nc.sync.dma_start(out=gate_sb, in_=_bcast(gate, DATA))
            w1t_sb = const.tile([DATA, WIDTH], BF16)
            nc.sync.dma_start(out=w1t_sb, in_=w1t[:, :])
            w2t_sb = const.tile([P, 2 * WIDTH], BF16)
            nc.sync.dma_start(out=w2t_sb, in_=w2t[:, :])
            w3t_sb = const.tile([P, 2 * WIDTH], BF16)
            nc.sync.dma_start(out=w3t_sb, in_=w3t[:, :])
            w4t_sb = const.tile([P, 2 * DATA], BF16)
            nc.sync.dma_start(out=w4t_sb, in_=w4t[:, :])
            b1_sb = const.tile([P, 2], F32)
            nc.sync.dma_start(out=b1_sb, in_=b1[:, :])
            b2_sb = const.tile([P, 2], F32)
            nc.sync.dma_start(out=b2_sb, in_=b2[:, :])
            b3_sb = const.tile([P, 2], F32)
            nc.sync.dma_start(out=b3_sb, in_=b3[:, :])
            b4_sb = const.tile([DATA, 1], F32)
            nc.sync.dma_start(out=b4_sb, in_=b4[:, :])

            # ---------------- phase 1: deltaT = ctx @ H^T ----------------
            with tc.tile_pool(name="htp", bufs=4) as htp, \
                 tc.tile_pool(name="p1ps", bufs=4, space="PSUM") as p1ps, \
                 tc.tile_pool(name="p1cp", bufs=4) as p1cp:
                for j in range(NBLK):
                    htt = htp.tile([CTXD, BLK], BF16)
                    nc.sync.dma_start(out=htt, in_=ht[j])
                    ps = p1ps.tile([P, 512], F32)
                    for g in range(4):
                        nc.tensor.matmul(
                            ps[32 * g:32 * g + ENVS, :],
                            lhsT=ctx_sb,
                            rhs=htt[:, g * 512:(g + 1) * 512],
                            start=True, stop=True,
                            tile_position=(0, 32 * g),
                        )
                    cp = p1cp.tile([P, 512], BF16)
                    nc.vector.tensor_copy(out=cp, in_=ps)
                    for g in range(4):
                        nc.sync.dma_start(
                            out=deltaT[j][:, g * 512:(g + 1) * 512],
                            in_=cp[32 * g:32 * g + ENVS, :],
                        )

            if DBG:
                nc.sync.dma_start(out=dbg_delta[:, :], in_=deltaT[8])

            # ---------------- phase 2: per-env MLP ----------------
            with tc.tile_pool(name="fw", bufs=2) as fwp, \
                 tc.tile_pool(name="biasp", bufs=2) as biasp, \
                 tc.tile_pool(name="ypool", bufs=2) as ypool, \
                 tc.tile_pool(name="hpool", bufs=6) as hpool, \
                 tc.tile_pool(name="opool", bufs=2) as opool, \
                 tc.tile_pool(name="psp", bufs=2, space="PSUM") as psp:
                for env in range(ENVS):
                    # -- modulated weights: fW = Wbase + delta chunk (bf16) --
                    # W1 region: blocks [0,8), flat = p*256 + w
                    dw1 = fwp.tile([DATA, WIDTH], BF16, tag="dw1")
                    nc.sync.dma_start(
                        out=dw1,
                        in_=deltaT[0:8, env, :].rearrange("a (q v) -> a q v", q=8))
                    fw1 = fwp.tile([DATA, WIDTH], BF16, tag="fw1")
                    nc.vector.tensor_add(out=fw1, in0=w1t_sb, in1=dw1)

                    # W2 region: blocks [8,40), flat = (kk*128 + 8a+q)*256 + v
                    dw2 = fwp.tile([P, 2 * WIDTH], BF16, tag="dw2")
                    for kk in range(2):
                        nc.sync.dma_start(
                            out=dw2[:, kk * WIDTH:(kk + 1) * WIDTH],
                            in_=deltaT[8 + 16 * kk:8 + 16 * (kk + 1), env, :]
                            .rearrange("a (q v) -> a q v", q=8))
                    fw2 = fwp.tile([P, 2 * WIDTH], BF16, tag="fw2")
                    nc.vector.tensor_add(out=fw2, in0=w2t_sb, in1=dw2)
                    if DBG and env == 0:
                        nc.sync.dma_start(out=dbg_dw2[:, :], in_=dw2)
                        nc.sync.dma_start(out=dbg_fw2[:, :], in_=fw2)

                    # W3 region: blocks [40,72)
                    dw3 = fwp.tile([P, 2 * WIDTH], BF16, tag="dw3")
                    for kk in range(2):
                        nc.sync.dma_start(
                            out=dw3[:, kk * WIDTH:(kk + 1) * WIDTH],
                            in_=deltaT[40 + 16 * kk:40 + 16 * (kk + 1), env, :]
                            .rearrange("a (q v) -> a q v", q=8))
                    fw3 = fwp.tile([P, 2 * WIDTH], BF16, tag="fw3")
                    nc.vector.tensor_add(out=fw3, in0=w3t_sb, in1=dw3)

                    # W4 region: blocks [72,80), flat = (kk*128 + 32a+q)*64 + d
                    dw4 = fwp.tile([P, 2 * DATA], BF16, tag="dw4")
                    for kk in range(2):
                        nc.sync.dma_start(
                            out=dw4[:, kk * DATA:(kk + 1) * DATA],
                            in_=deltaT[72 + 4 * kk:72 + 4 * (kk + 1), env, :]
                            .rearrange("a (q d) -> a q d", q=32))
                    fw4 = fwp.tile([P, 2 * DATA], BF16, tag="fw4")
                    nc.vector.tensor_add(out=fw4, in0=w4t_sb, in1=dw4)

                    # biases live in block 80, cols [0, 832)
                    db1 = biasp.tile([P, 2], BF16, tag="db1")
                    nc.sync.dma_start(
                        out=db1,
                        in_=deltaT[80, env, O_B1 - 80 * BLK:O_B2 - 80 * BLK]
                        .rearrange("(mt p) -> p mt", mt=2))
                    fb1 = biasp.tile([P, 2], F32, tag="fb1")
                    nc.vector.tensor_add(out=fb1, in0=b1_sb, in1=db1)

                    db2 = biasp.tile([P, 2], BF16, tag="db2")
                    nc.sync.dma_start(
                        out=db2,
                        in_=deltaT[80, env, O_B2 - 80 * BLK:O_B3 - 80 * BLK]
                        .rearrange("(mt p) -> p mt", mt=2))
                    fb2 = biasp.tile([P, 2], F32, tag="fb2")
                    nc.vector.tensor_add(out=fb2, in0=b2_sb, in1=db2)

                    db3 = biasp.tile([P, 2], BF16, tag="db3")
                    nc.sync.dma_start(
                        out=db3,
                        in_=deltaT[80, env, O_B3 - 80 * BLK:O_B4 - 80 * BLK]
                        .rearrange("(mt p) -> p mt", mt=2))
                    fb3 = biasp.tile([P, 2], F32, tag="fb3")
                    nc.vector.tensor_add(out=fb3, in0=b3_sb, in1=db3)

                    db4 = biasp.tile([DATA, 1], BF16, tag="db4")
                    nc.sync.dma_start(
                        out=db4,
                        in_=deltaT[80, env, O_B4 - 80 * BLK:O_B4 - 80 * BLK + DATA]
                        .rearrange("(p one) -> p one", one=1))
                    fb4g = biasp.tile([DATA, 1], F32, tag="fb4g")
                    nc.vector.tensor_add(out=fb4g, in0=b4_sb, in1=db4)
                    # fold gate into bias: fb4g = (b4 + db4) * gate[env]
                    nc.vector.tensor_mul(out=fb4g, in0=fb4g,
                                         in1=gate_sb[:, env:env + 1])

                    ysb = ypool.tile([DATA, NPTS], BF16)
                    nc.sync.dma_start(out=ysb, in_=yt[env])

                    # -- layer 1: h1 = silu(beta*(W1f @ y) + beta*b1f) --
                    h1 = []
                    ps1 = [psp.tile([P, NPTS], F32, tag="ps", name=f"ps1_{env}_{i}")
                           for i in range(2)]
                    for mt in range(2):
                        for t in range(4):
                            nc.tensor.matmul(
                                ps1[mt][:, t * 512:(t + 1) * 512],
                                lhsT=fw1[:, mt * P:(mt + 1) * P],
                                rhs=ysb[:, t * 512:(t + 1) * 512],
                                start=True, stop=True)
                        ht1 = hpool.tile([P, NPTS], BF16, tag="h")
                        nc.scalar.activation(
                            out=ht1, in_=ps1[mt][:, :], func=SILU,
                            bias=fb1[:, mt:mt + 1], scale=beta_sb[:, 0:1])
                        h1.append(ht1)

                    if DBG and env == 0:
                        nc.sync.dma_start(out=dbg_y[:, :], in_=ysb)
                        nc.sync.dma_start(out=dbg_h1[:, :], in_=h1[0])

                    # -- layers 2/3 --
                    hprev = h1
                    for li, (fw_l, fb_l) in enumerate(((fw2, fb2), (fw3, fb3))):
                        hcur = []
                        psl = [psp.tile([P, NPTS], F32, tag="ps",
                                        name=f"psl_{env}_{li}_{i}")
                               for i in range(2)]
                        for mm in range(2):
                            for kk in range(2):
                                for t in range(4):
                                    nc.tensor.matmul(
                                        psl[mm][:, t * 512:(t + 1) * 512],
                                        lhsT=fw_l[:, kk * WIDTH + mm * P:
                                                  kk * WIDTH + (mm + 1) * P],
                                        rhs=hprev[kk][:, t * 512:(t + 1) * 512],
                                        start=(kk == 0), stop=(kk == 1))
                            htl = hpool.tile([P, NPTS], BF16, tag="h")
                            nc.scalar.activation(
                                out=htl, in_=psl[mm][:, :], func=SILU,
                                bias=fb_l[:, mm:mm + 1], scale=beta_sb[:, 0:1])
                            hcur.append(htl)
                        hprev = hcur

                    # -- layer 4 + epilogue --
                    ps4 = psp.tile([DATA, NPTS], F32, tag="ps")
                    for kk in range(2):
                        for t in range(4):
                            nc.tensor.matmul(
                                ps4[:, t * 512:(t + 1) * 512],
                                lhsT=fw4[:, kk * DATA:(kk + 1) * DATA],
                                rhs=hprev[kk][:, t * 512:(t + 1) * 512],
                                start=(kk == 0), stop=(kk == 1))
                    osb = opool.tile([DATA, NPTS], F32)
                    # out = gate*ps4 + gate*(b4+db4)
                    nc.vector.tensor_scalar(
                        out=osb, in0=ps4[:, :],
                        scalar1=gate_sb[:DATA, env:env + 1], scalar2=fb4g[:, 0:1],
                        op0=MULT, op1=ADD)
                    nc.sync.dma_start(out=out[env], in_=osb)

    nc.compile()
    _NC_CACHE["nc"] = nc
    return nc


# --------------------------------------------------------------------------
# entry point
# --------------------------------------------------------------------------
def kernel(t, y, ctx, W1, b1, W2, b2, W3, b3, W4, b4, H, G, beta):
    global LAST_RESULTS
    y = np.asarray(y, np.float32)
    ctx = np.asarray(ctx, np.float32)
    H = np.asarray(H, np.float32)
    G = np.asarray(G, np.float32)
    beta = np.asarray(beta, np.float32)
    W = [np.asarray(w, np.float32) for w in (W1, W2, W3, W4)]
    b = [np.asarray(x, np.float32) for x in (b1, b2, b3, b4)]

    in_maps = _prep_inputs(y, ctx, W, b, H, G, beta)
    nc = _build_nc()
    res = run_bass_kernel_spmd(
        nc, in_maps, list(range(N_CORES)),
        trace=TRACE, trace_cores=None)
    LAST_RESULTS = res

    total = np.zeros((ENVS, DATA, NPTS), np.float32)
    for e in range(N_CORES):
        total += res.results[e]["out"]
    return np.ascontiguousarray(total.transpose(0, 2, 1))


def measure_exec_ns(inputs, iters=64, warmup=4):
    """Steady-state per-execution time of the compiled NEFF on 8 cores.

    Keeps inputs device-resident and measures the marginal wall time of
    pipelined executions. The result still contains per-call dispatch
    overhead (compare against a trivial kernel's floor for the difference).
    Used by test.py only; the grading path never calls this.
    """
    import time
    import jax
    from jax.sharding import Mesh, PartitionSpec, NamedSharding
    from jax.experimental.shard_map import shard_map
    from concourse import bass2jax, mybir as _mybir

    y = np.asarray(inputs["y"], np.float32)
    ctx = np.asarray(inputs["ctx"], np.float32)
    H = np.asarray(inputs["H"], np.float32)
    G = np.asarray(inputs["G"], np.float32)
    beta = np.asarray(inputs["beta"], np.float32)
    W = [np.asarray(inputs[k], np.float32) for k in ("W1", "W2", "W3", "W4")]
    b = [np.asarray(inputs[k], np.float32) for k in ("b1", "b2", "b3", "b4")]
    in_maps = _prep_inputs(y, ctx, W, b, H, G, beta)
    nc = _build_nc()

    bass2jax.install_neuronx_cc_hook()
    partition_name = nc.partition_id_tensor.name if nc.partition_id_tensor else None
    in_names, out_names, out_avals, zero_outs = [], [], [], []
    for alloc in nc.m.functions[0].allocations:
        if not isinstance(alloc, _mybir.MemoryLocationSet):
            continue
        name = alloc.memorylocations[0].name
        if alloc.kind == "ExternalInput":
            if name != partition_name:
                in_names.append(name)
        elif alloc.kind == "ExternalOutput":
            shape = tuple(alloc.tensor_shape)
            dtype = _mybir.dt.np(alloc.dtype)
            out_names.append(name)
            out_avals.append(jax.core.ShapedArray(shape, dtype))
            zero_outs.append(np.zeros(shape, dtype))
    n_params = len(in_names)
    all_in_names = in_names + out_names
    if partition_name is not None:
        all_in_names.append(partition_name)

    def _body(*args):
        operands = list(args)
        if partition_name is not None:
            operands.append(bass2jax.partition_id_tensor())
        outs = bass2jax._bass_exec_p.bind(
            *operands,
            out_avals=tuple(out_avals),
            in_names=tuple(all_in_names),
            out_names=tuple(out_names),
            lowering_input_output_aliases=(),
            sim_require_finite=True,
            sim_require_nnan=True,
            nc=nc,
        )
        return tuple(outs)

    devices = jax.devices()[:N_CORES]
    mesh = Mesh(np.asarray(devices), ("core",))
    nspec = NamedSharding(mesh, PartitionSpec("core"))
    n_all = n_params + len(out_names)
    sharded = jax.jit(
        shard_map(_body, mesh=mesh,
                  in_specs=(PartitionSpec("core"),) * n_all,
                  out_specs=(PartitionSpec("core"),) * len(out_names),
                  check_rep=False),
        keep_unused=True)

    concat_in = [
        np.concatenate([np.asarray(in_maps[c][k]) for c in range(N_CORES)], axis=0)
        for k in in_names
    ] + [np.zeros((N_CORES * z.shape[0], *z.shape[1:]), z.dtype) for z in zero_outs]
    dev_in = [jax.device_put(a, nspec) for a in concat_in]

    for _ in range(warmup):
        outs = sharded(*dev_in)
    jax.block_until_ready(outs)

    t0 = time.perf_counter()
    for _ in range(iters):
        outs = sharded(*dev_in)
    jax.block_until_ready(outs)
    t1 = time.perf_counter()
    per_call = (t1 - t0) / iters

    return {"pipelined_ns": per_call * 1e9}


if __name__ == "__main__":
    _build_nc()
    print("IR build OK")

